# revision 8
# baseline (speedup 1.0000x reference)
"""Trainium2 Bass kernel for ExpBertSelfAttention (B=2, S=2048, D=1024, H=16).

Sharding: 8 cores; core c handles batch b=c//4 and 4 consecutive heads
4*(c%4)..4*(c%4)+3 (data-parallel on B, tensor-parallel on heads).  The dense
output projection is row-parallel, so each core returns a partial [S, D] sum
in bf16; the host sums the 4 partials per batch and adds b_dense.

v3: all-bf16 pipeline (fp8 matmuls measure ~2-3% output error per quantized
tensor -- over the 2e-2 gate -- so 16-bit everywhere; bf16 total err ~4e-3).
Per-core structure:
  - qkvT [128, 4(Qp0,Qp1,Kp0,Kp1), S] bf16: head pair p on partition halves
    (head h at partitions 64*(h%2)..+64 of pair tile h//2), 1/sqrt(64)
    folded into Wq host-side.
  - V projected directly in [seq, feat] orientation (hsT stationary), V
    bias injected with a rank-1 ones-row matmul; v_sb [128, h, kt, 65] with
    a ones column at 64 so the PV matmul emits softmax row sums for free.
  - scores per (head, kt, q-chunk): single k=64 matmul (stationary K tile
    at base partition 0/64); exp on ACT reads [128, 1024] PSUM, writes bf16
    probs; multiplicative {0,1} bf16 mask applied by DVE in-place on the
    probs (2x DVE mode; exact zeros).  Mask loaded once per q-chunk.
  - normalize: DVE copies the PSUM rowsum row to SBUF f32r, a k=1 f32r
    matmul broadcasts it over 64 partitions, DVE reciprocal + multiply
    (PSUM x SBUF -> bf16 ctx); odd heads staged via SBUF->SBUF DMA to
    partitions 64-127 of ctx_pair.
  - dense per q-chunk right after its 4 heads finish (overlaps the next
    chunk's attention): 2-step k=128 accumulation, PSUM -> bf16 copies
    split across ACT and DVE, 16 output DMAs.
"""

import os
import sys

for _p in ("/opt/trn_rl_repo", "/root/.axon_site/_ro/trn_rl_repo"):
    if os.path.isdir(_p) and _p not in sys.path:
        sys.path.insert(0, _p)

import numpy as np
import ml_dtypes

import concourse.bass as bass
import concourse.tile as tile
from concourse import bacc, mybir
from concourse import bass_utils

B, S, D, H = 2, 2048, 1024, 16
HD = D // H  # 64
SCALE = 8.0  # sqrt(HD)
NCORES = 8
HPC = H // (NCORES // B)  # heads per core = 4
P = 128
KT_S = S // P  # 16 key tiles

F32 = mybir.dt.float32
F32R = mybir.dt.float32r
BF16 = mybir.dt.bfloat16
AF = mybir.ActivationFunctionType
MUL = mybir.AluOpType.mult


def build_program():
    nc = bacc.Bacc("TRN2", target_bir_lowering=False, debug=False,
                   num_devices=NCORES)

    hsT = nc.dram_tensor("hsT", [D, S], BF16, kind="ExternalInput").ap()
    wqkv = nc.dram_tensor("wqkv", [D, 6 * P], BF16, kind="ExternalInput").ap()
    bqk = nc.dram_tensor("bqk", [P, 4], F32, kind="ExternalInput").ap()
    wbv = nc.dram_tensor("wbv", [1, 2 * P], BF16, kind="ExternalInput").ap()
    maskdr = nc.dram_tensor("maskdr", [S, S], BF16, kind="ExternalInput").ap()
    wd = nc.dram_tensor("wd", [2 * P, D], BF16, kind="ExternalInput").ap()
    y = nc.dram_tensor("y", [S, D], BF16, kind="ExternalOutput").ap()

    with tile.TileContext(nc) as tc:
        with tc.tile_pool(name="persist", bufs=1) as persist:
            qkvT = persist.tile([P, 4, S], BF16)           # 16 KB/part
            v_sb = persist.tile([P, HPC, KT_S, HD + 1], BF16)
            ctx_pair = persist.tile([P, 2, S], BF16)       # 8 KB/part
            wd_sb = persist.tile([P, 2, D], BF16)          # 4 KB/part
            bqk_sb = persist.tile([P, 4], F32)
            mask_sb = [persist.tile([P, KT_S, 1024], BF16,
                                    name=f"mask{qc}") for qc in range(2)]
            ones32 = persist.tile([1, HD], F32R)
            hs1 = persist.tile([1, P], BF16)
            wbv_sb = persist.tile([1, 2 * P], BF16)

            o32f = persist.tile([1, HD], F32)
            nc.vector.memset(o32f[:], 1.0)
            nc.vector.tensor_copy(ones32[:], o32f[:])
            hs1f = persist.tile([1, P], F32)
            nc.vector.memset(hs1f[:], 1.0)
            nc.vector.tensor_copy(hs1[:], hs1f[:])
            onesvf = persist.tile([P, KT_S], F32)
            nc.vector.memset(onesvf[:], 1.0)
            for h in range(HPC):
                nc.vector.tensor_copy(
                    v_sb[:, h, :, HD:HD + 1].rearrange("p k one -> p (k one)"),
                    onesvf[:])

            nc.sync.dma_start(wd_sb[:], wd.rearrange("(t p) n -> p t n", p=P))
            nc.sync.dma_start(bqk_sb[:], bqk)
            nc.sync.dma_start(wbv_sb[:], wbv)
            # mask loads issued early; 4 kt-groups per q-chunk
            for qc in range(2):
                for g in range(4):
                    nc.sync.dma_start(
                        mask_sb[qc][:, 4 * g:4 * g + 4, :],
                        maskdr[g * 512:(g + 1) * 512,
                               qc * 1024:(qc + 1) * 1024].rearrange(
                                   "(kt p) q -> p kt q", p=P))

            # ---------------- Phase 1: QKV projection ----------------
            with (
                tc.tile_pool(name="p1sb", bufs=1) as p1sb,
                tc.tile_pool(name="qkps", bufs=4, space="PSUM") as qkps,
                tc.tile_pool(name="vtps", bufs=2, space="PSUM") as vtps,
            ):
                hsT_sb = p1sb.tile([P, 8, S], BF16)          # 32 KB/part
                wqkv_sb = p1sb.tile([P, 8, 6 * P], BF16)     # 12 KB/part
                hsT_r = hsT.rearrange("(t p) n -> p t n", p=P)
                nc.sync.dma_start(wqkv_sb[:],
                                  wqkv.rearrange("(t p) n -> p t n", p=P))
                for nch in range(4):
                    nc.sync.dma_start(
                        hsT_sb[:, :, nch * 512:(nch + 1) * 512],
                        hsT_r[:, :, nch * 512:(nch + 1) * 512])
                for nch in range(4):
                    ns = slice(nch * 512, (nch + 1) * 512)
                    ps_l = [qkps.tile([P, 512], F32, tag="qk",
                                      name=f"qk{nch}_{mt}")
                            for mt in range(4)]
                    for ktp in range(8):
                        for mt in range(4):
                            nc.tensor.matmul(
                                ps_l[mt][:],
                                wqkv_sb[:, ktp, mt * P:(mt + 1) * P],
                                hsT_sb[:, ktp, ns],
                                start=(ktp == 0), stop=(ktp == 7))
                    for mt in range(4):
                        nc.vector.tensor_scalar_add(
                            qkvT[:, mt, ns], ps_l[mt][:],
                            bqk_sb[:, mt:mt + 1])
                    # V directly in [seq, feat] orientation
                    for kti in range(4):
                        kt = nch * 4 + kti
                        for half in range(2):
                            vt = vtps.tile([P, P], F32, tag="vt")
                            for ktp in range(8):
                                nc.tensor.matmul(
                                    vt[:],
                                    hsT_sb[:, ktp, kt * P:(kt + 1) * P],
                                    wqkv_sb[:, ktp,
                                            (4 + half) * P:(5 + half) * P],
                                    start=(ktp == 0), stop=False)
                            nc.tensor.matmul(
                                vt[:], hs1[:],
                                wbv_sb[:, half * P:(half + 1) * P],
                                start=False, stop=True,
                                skip_group_check=True)
                            nc.vector.tensor_copy(
                                v_sb[:, 2 * half:2 * half + 2, kt, 0:HD],
                                vt[:].rearrange("p (h j) -> p h j", h=2))

            # ---------------- Phase 2: attention ----------------
            with (
                tc.tile_pool(name="pp", bufs=6) as pp,
                tc.tile_pool(name="np_", bufs=2) as np_,
                tc.tile_pool(name="yp", bufs=4) as yp,
                tc.tile_pool(name="sps", bufs=2, space="PSUM") as sps,
                tc.tile_pool(name="cps", bufs=2, space="PSUM") as cps,
            ):
                for qc in range(2):
                    q0 = qc * 1024
                    for h in range(HPC):
                        hb = 64 * (h % 2)   # partition base of this head
                        pr = h // 2         # pair tile index
                        ctx = cps.tile([HD + 1, 1024], F32, tag="ctx")
                        for kt in range(KT_S):
                            s_ps = sps.tile([P, 1024], F32, tag="s")
                            for ch in range(2):
                                cs = slice(ch * 512, (ch + 1) * 512)
                                nc.tensor.matmul(
                                    s_ps[:, cs],
                                    qkvT[hb:hb + HD, 2 + pr,
                                         kt * P:(kt + 1) * P],
                                    qkvT[hb:hb + HD, 0 + pr,
                                         q0 + ch * 512:q0 + (ch + 1) * 512],
                                    start=True, stop=True)
                            prt = pp.tile([P, 1024], BF16, tag="probs")
                            nc.scalar.activation(prt[:], s_ps[:], AF.Exp)
                            nc.vector.tensor_tensor(
                                prt[:], prt[:], mask_sb[qc][:, kt, :], op=MUL)
                            for ch in range(2):
                                cs = slice(ch * 512, (ch + 1) * 512)
                                nc.tensor.matmul(
                                    ctx[:, cs],
                                    v_sb[:, h, kt, :],
                                    prt[:, cs],
                                    start=(kt == 0), stop=(kt == KT_S - 1))
                        # normalize: rowsum is PSUM row 64 (ones col of v_sb)
                        rrow = np_.tile([1, 1024], F32R, tag="rrow")
                        nc.vector.tensor_copy(rrow[:], ctx[HD:HD + 1, :])
                        rb = sps.tile([P, 1024], F32, tag="s",
                                      name=f"rb{qc}_{h}")
                        for ch in range(2):
                            cs = slice(ch * 512, (ch + 1) * 512)
                            nc.tensor.matmul(rb[0:HD, cs], ones32[:],
                                             rrow[:, cs],
                                             start=True, stop=True)
                        rbi = np_.tile([HD, 1024], F32, tag="rbi")
                        nc.vector.reciprocal_approx_fast(rbi[:], rb[0:HD, :])
                        if h % 2 == 0:
                            nc.vector.tensor_tensor(
                                ctx_pair[0:HD, pr, q0:q0 + 1024],
                                ctx[0:HD, :], rbi[:], op=MUL)
                        else:
                            stg = np_.tile([HD, 1024], BF16, tag="stg")
                            nc.vector.tensor_tensor(stg[:], ctx[0:HD, :],
                                                    rbi[:], op=MUL)
                            nc.sync.dma_start(
                                ctx_pair[HD:P, pr, q0:q0 + 1024], stg[:])
                    # dense for this q-chunk (overlaps next chunk's attention)
                    for mti in range(8):
                        mt = qc * 8 + mti
                        dp = sps.tile([P, 1024], F32, tag="s",
                                      name=f"d{qc}_{mti}")
                        for nch in range(2):
                            ncs = slice(nch * 512, (nch + 1) * 512)
                            for t in range(2):
                                nc.tensor.matmul(
                                    dp[:, ncs],
                                    ctx_pair[:, t, mt * P:(mt + 1) * P],
                                    wd_sb[:, t, ncs],
                                    start=(t == 0), stop=(t == 1))
                        ysb = yp.tile([P, D], BF16, tag="y")
                        nc.scalar.copy(ysb[:, 0:512], dp[:, 0:512])
                        nc.vector.tensor_copy(ysb[:, 512:1024],
                                              dp[:, 512:1024])
                        nc.sync.dma_start(y[mt * P:(mt + 1) * P, :], ysb[:])

    nc.compile()
    return nc


_NC = None


def get_program():
    global _NC
    if _NC is None:
        _NC = build_program()
    return _NC


def make_in_maps(hidden_states, attention_mask, W_qkv, b_qkv, W_dense,
                 b_dense):
    hs = np.asarray(hidden_states, np.float32)
    mask = np.asarray(attention_mask)
    W_qkv = np.asarray(W_qkv, np.float32)
    b_qkv = np.asarray(b_qkv, np.float32)
    W_dense = np.asarray(W_dense, np.float32)

    BFNP = ml_dtypes.bfloat16

    hsT = [np.ascontiguousarray(hs[b].T).astype(BFNP) for b in range(B)]
    maskT = [np.ascontiguousarray(
        np.where(mask[b, 0], 1.0, 0.0).astype(np.float32).T
    ).astype(BFNP) for b in range(B)]

    Wq, Wk, Wv = W_qkv[:, :D], W_qkv[:, D:2 * D], W_qkv[:, 2 * D:]
    bq, bk, bv = b_qkv[:D], b_qkv[D:2 * D], b_qkv[2 * D:]

    in_maps = []
    for c in range(NCORES):
        b = c // (NCORES // B)
        h0 = HPC * (c % (NCORES // B))
        cols0 = slice((h0 + 0) * HD, (h0 + 2) * HD)  # pair 0: heads 0,1
        cols1 = slice((h0 + 2) * HD, (h0 + 4) * HD)  # pair 1: heads 2,3
        # m-tiles [Qp0, Qp1, Kp0, Kp1, Vp0, Vp1]; 1/SCALE folded into Wq
        wqkv_c = np.ascontiguousarray(np.concatenate([
            Wq[:, cols0] / SCALE, Wq[:, cols1] / SCALE,
            Wk[:, cols0], Wk[:, cols1],
            Wv[:, cols0], Wv[:, cols1]], axis=1)).astype(BFNP)
        bqk_c = np.concatenate([
            bq[cols0] / SCALE, bq[cols1] / SCALE,
            bk[cols0], bk[cols1]]).reshape(4, P).T.astype(np.float32)
        wbv_c = np.concatenate([bv[cols0], bv[cols1]])[None, :].astype(BFNP)
        wd_c = np.ascontiguousarray(
            W_dense[h0 * HD:(h0 + HPC) * HD, :]).astype(BFNP)
        in_maps.append({
            "hsT": hsT[b],
            "wqkv": wqkv_c,
            "bqk": bqk_c,
            "wbv": wbv_c,
            "maskdr": maskT[b],
            "wd": wd_c,
        })
    return in_maps


def kernel(hidden_states, attention_mask, W_qkv, b_qkv, W_dense, b_dense,
           **run_kwargs):
    nc = get_program()
    in_maps = make_in_maps(hidden_states, attention_mask, W_qkv, b_qkv,
                           W_dense, b_dense)
    res = bass_utils.run_bass_kernel_spmd(
        nc, in_maps, core_ids=list(range(NCORES)), **run_kwargs)
    out = np.zeros((B, S, D), np.float32)
    gpb = NCORES // B
    for c in range(NCORES):
        out[c // gpb] += res.results[c]["y"].astype(np.float32)
    out += np.asarray(b_dense, np.float32)
    if run_kwargs:
        kernel.last_results = res
    return out
